# revision 10
# baseline (speedup 1.0000x reference)
"""Trainium2 Bass kernel for depth-softmax attention over stacked slices.

Computes, for V[N=12, B=4, S=2048, D=2048] (fp32), norm_scale[D], query[D]:
    rms    = sqrt(mean_d(V^2) + 1e-6)                  # per (n, b, s)
    logits = einsum("nbsd,d->nbs", V / rms, norm_scale * query)
    w      = softmax(logits, axis=0)                   # over the 12 slices
    out    = einsum("nbs,nbsd->bsd", w, V)

Sharding: the flattened B*S = 8192 token positions are split evenly across
8 NeuronCores (1024 positions per core, contiguous in S).  norm_scale*query
is replicated (shipped pre-replicated across the 128 partitions).

V is shipped to the device as float16 (host-side cast during input packing).
The relative-error budget (2e-2) dwarfs the fp16 quantization error of the
whole pipeline (~1.3e-3, measured off-line on the reference distribution),
and halving the HBM read bytes halves the memory roofline: per core the
kernel reads 50.3 MB (V) + writes 8.4 MB (out, fp32).

Per-core dataflow (positions tiled 8 x 128, partition dim = positions):
  - DMA V[n] pair tiles [128, 2, 2048] fp16 into SBUF (1 MiB loads, SP HWDGE)
  - DVE: scalar_tensor_tensor (fp16, 2x-eligible) -> dot_n = sum_d V*wq
  - ssq_n = sum_d V^2: 9 slices on ScalarE (Square activation w/ accum),
    3 on DVE (STT with in0=in1=V), balancing the two engines
  - rrms = Newton rsqrt of (ssq/D + eps); logits = dot * rrms  (DVE, fp32)
  - softmax over the 12 logits held as a [128, 12] tile (free-dim reduce)
  - all 12 diag(w_n) matrices are built in ONE DVE op: dg[p,n,:] =
    id[p,:] * w[p,n] via broadcast APs (id with a stride-0 slice dim,
    expw with a stride-0 column dim)
  - TensorE: out_tile = sum_n diag(w_n) @ V_n accumulated in PSUM, fp16
    matmuls (1 cyc/row), bank-major
  - PSUM -> SBUF copy applies the 1/sum(exp) normalization as the ACT
    copy's scale; one contiguous 1 MiB store per tile (ACT HWDGE queue)

The Bass init all-engine barrier is skipped: it only fences the const-AP
memsets (gpsimd), and no instruction here reads a const AP -- activations
that need a zero bias use an explicitly DMA'd zeros tile instead.

Last tile uses the baseline's split softmax: provisional max over slices
0..9 lets 40 of 48 matmuls run before the final pair of slices arrives;
slices 10/11 use exp(min(l - M10, 80)) which is exact to fp32 precision
whenever the clamp engages (the clamped slice then dominates by >= e^17).
"""

import numpy as np

N_SLICES = 12
B = 4
S = 2048
D = 2048
NCORES = 8
POS_PER_CORE = (B * S) // NCORES  # 1024
TILE_P = 128
NTILES = POS_PER_CORE // TILE_P  # 8
DBLOCK = 512  # one PSUM bank of fp32
EPS = 1e-6

# ssq on DVE for these slices, ScalarE for the rest (load balancing).
SSQ_DVE_SET = (3, 5, 7)
NEWTON_STEPS = 2

_CACHE = {}

SKIP_INIT_BARRIER = True


def _build_module():
    from concourse import bacc, tile
    from concourse import bass as bass_mod
    import concourse.mybir as mybir

    f32 = mybir.dt.float32
    f16 = mybir.dt.float16
    AF = mybir.ActivationFunctionType
    OP = mybir.AluOpType

    if SKIP_INIT_BARRIER:
        orig_barrier = bass_mod.Bass.all_engine_barrier
        bass_mod.Bass.all_engine_barrier = lambda self, **kw: None
        try:
            nc = bacc.Bacc(
                "TRN2", target_bir_lowering=False, debug=False,
                enable_partition_id=False, detect_race_conditions=False,
            )
        finally:
            bass_mod.Bass.all_engine_barrier = orig_barrier
    else:
        nc = bacc.Bacc(
            "TRN2", target_bir_lowering=False, debug=False,
            enable_partition_id=False,
        )

    V = nc.dram_tensor("v_in", [N_SLICES, POS_PER_CORE, D], f16, kind="ExternalInput")
    WQ = nc.dram_tensor("wq_in", [TILE_P, D], f16, kind="ExternalInput")
    IDENT = nc.dram_tensor("id_in", [TILE_P, TILE_P], f16, kind="ExternalInput")
    ZB = nc.dram_tensor("zb_in", [TILE_P, 1], f32, kind="ExternalInput")
    OUT = nc.dram_tensor("out", [POS_PER_CORE, D], f32, kind="ExternalOutput")

    Vap, WQap, OUTap = V.ap(), WQ.ap(), OUT.ap()

    with tile.TileContext(nc) as tc:
        with (
            tc.tile_pool(name="consts", bufs=1) as consts,
            tc.tile_pool(name="vpool", bufs=16) as vpool,
            tc.tile_pool(name="scr", bufs=2) as scr,
            tc.tile_pool(name="stats", bufs=2) as stats,
            tc.tile_pool(name="outp", bufs=2) as outp,
            tc.tile_pool(name="psum", bufs=2, space="PSUM") as psump,
        ):
            # Const loads on the ACT HWDGE queue; V stream on the SP queue.
            wq_sb = consts.tile([TILE_P, D], f16, tag="wq")
            nc.scalar.dma_start(out=wq_sb[:], in_=WQap[:, :])
            zb = consts.tile([TILE_P, 1], f32, tag="zb")
            nc.scalar.dma_start(out=zb[:], in_=ZB.ap()[:, :])
            id_sb = consts.tile([TILE_P, TILE_P], f16, tag="ident")
            nc.scalar.dma_start(out=id_sb[:], in_=IDENT.ap()[:, :])
            id_bc = id_sb[:].rearrange("p (n d) -> p n d", n=1)

            def emit_diags(dg, w_tile, lo, hi):
                # dg[p, lo:hi, :] = id[p, :] * w[p, n] -- one DVE op builds
                # all the diag matrices for slices lo..hi-1.
                width = hi - lo
                nc.vector.scalar_tensor_tensor(
                    out=dg[:, lo:hi, :],
                    in0=id_bc.to_broadcast((TILE_P, width, TILE_P)),
                    scalar=1.0,
                    in1=w_tile[:].rearrange("p (n d) -> p n d", d=1)
                        .to_broadcast((TILE_P, width, TILE_P)),
                    op0=OP.mult,
                    op1=OP.mult,
                )

            for t in range(NTILES):
                p0 = t * TILE_P
                dg = stats.tile([TILE_P, N_SLICES, TILE_P], f16, tag="dg")
                vtiles = []
                ssq = stats.tile([TILE_P, N_SLICES], f32, tag="ssq")
                dot = stats.tile([TILE_P, N_SLICES], f32, tag="dot")
                last_tile = t == NTILES - 1

                def emit_pair_dma(pair, split):
                    # One 1 MiB DMA covers two fp16 depth slices (4 KiB
                    # contiguous rows).  The last pair of the last tile is
                    # split into two 0.5 MiB loads so the final slice's
                    # reduction starts as early as possible.
                    vb2 = vpool.tile([TILE_P, 2, D], f16, tag="vb")
                    src = Vap[2 * pair : 2 * pair + 2, p0 : p0 + TILE_P, :]
                    if split:
                        nc.sync.dma_start(out=vb2[:, 0, :], in_=src[0])
                        nc.sync.dma_start(out=vb2[:, 1, :], in_=src[1])
                    else:
                        nc.sync.dma_start(
                            out=vb2[:], in_=src.rearrange("n p d -> p n d")
                        )
                    vtiles.append(vb2)

                def emit_phase1(n):
                    vb = vtiles[n // 2][:, n % 2, :]
                    # dot[p] = sum_d V[p,d]*WQ[d] on DVE; fp16 in/out so the
                    # 2x perf mode can engage (out must be a real packed
                    # tile, not a broadcast AP).
                    dot_scr = scr.tile([TILE_P, D], f16, tag="dot_scr")
                    nc.vector.scalar_tensor_tensor(
                        out=dot_scr[:],
                        in0=vb,
                        scalar=1.0,
                        in1=wq_sb[:],
                        op0=OP.mult,
                        op1=OP.mult,
                        accum_out=dot[:, n : n + 1],
                    )
                    if n in SSQ_DVE_SET:
                        sq_scr = scr.tile([TILE_P, D], f16, tag="sq_scr")
                        nc.vector.scalar_tensor_tensor(
                            out=sq_scr[:],
                            in0=vb,
                            scalar=1.0,
                            in1=vb,
                            op0=OP.mult,
                            op1=OP.mult,
                            accum_out=ssq[:, n : n + 1],
                        )
                    else:
                        # ScalarE Square with accumulate; main output goes to
                        # a stride-0 broadcast scrap.  bias is an explicit
                        # zeros tile (not the const AP -- the init barrier
                        # that fences const-AP memsets is skipped).
                        act_scr = scr.tile([TILE_P, 1], f32, tag="act_scr")
                        nc.scalar.activation(
                            act_scr[:].to_broadcast((TILE_P, D)), vb, AF.Square,
                            bias=zb[:], accum_out=ssq[:, n : n + 1],
                        )

                n_early = (N_SLICES - 2) if last_tile else N_SLICES
                for pair in range(n_early // 2):
                    emit_pair_dma(pair, split=False)
                for n in range(n_early):
                    emit_phase1(n)

                # logits = dot * rsqrt(ssq/D + eps); Newton rsqrt (msq is
                # within ~16% of 1.0, y0 + 2 steps -> ~3e-7 rel err).
                def emit_logits(lo, hi, sfx):
                    width = hi - lo
                    msq = stats.tile([TILE_P, width], f32, tag=f"msq{sfx}")
                    nc.vector.tensor_scalar(
                        out=msq[:], in0=ssq[:, lo:hi], scalar1=1.0 / D,
                        scalar2=EPS, op0=OP.mult, op1=OP.add,
                    )
                    y = stats.tile([TILE_P, width], f32, tag=f"nwt_y{sfx}")
                    nc.vector.tensor_scalar(
                        out=y[:], in0=msq[:], scalar1=-0.5, scalar2=1.5,
                        op0=OP.mult, op1=OP.add,
                    )
                    for it in range(NEWTON_STEPS):
                        t1 = stats.tile([TILE_P, width], f32, tag=f"nwt_t{it}{sfx}")
                        nc.vector.tensor_mul(t1[:], y[:], y[:])
                        nc.vector.tensor_mul(t1[:], t1[:], msq[:])
                        nc.vector.tensor_scalar(
                            out=t1[:], in0=t1[:], scalar1=-0.5, scalar2=1.5,
                            op0=OP.mult, op1=OP.add,
                        )
                        y2 = stats.tile([TILE_P, width], f32, tag=f"nwt_y{it}{sfx}")
                        nc.vector.tensor_mul(y2[:], y[:], t1[:])
                        y = y2
                    logits = stats.tile([TILE_P, width], f32, tag=f"logits{sfx}")
                    nc.vector.tensor_mul(logits[:], dot[:, lo:hi], y[:])
                    return logits

                logitsA = emit_logits(0, n_early, "A")
                negmax = stats.tile([TILE_P, 1], f32, tag="negmax")
                nc.vector.tensor_reduce(
                    negmax[:], logitsA[:], axis=mybir.AxisListType.X,
                    op=OP.max, negate=True,
                )
                # Unnormalized weights exp(l - max); 1/sum(exp) is applied
                # later as the PSUM->SBUF copy's per-partition scale.
                expw = stats.tile([TILE_P, n_early], f32, tag="expw")
                sumexp = stats.tile([TILE_P, 1], f32, tag="sumexp")
                nc.scalar.activation(
                    expw[:], logitsA[:], AF.Exp, bias=negmax[:],
                    accum_out=sumexp[:],
                )
                emit_diags(dg, expw, 0, n_early)

                ps = psump.tile([TILE_P, D], f32, tag="ps")
                o_sb = outp.tile([TILE_P, D], f32, tag="o_sb")

                def emit_copy(blk, rsum, on_dve):
                    if on_dve:
                        nc.vector.tensor_scalar(
                            out=o_sb[:, blk], in0=ps[:, blk], scalar1=rsum[:],
                            scalar2=None, op0=OP.mult,
                        )
                    else:
                        nc.scalar.activation(
                            o_sb[:, blk], ps[:, blk], AF.Copy, scale=rsum[:]
                        )

                if not last_tile:
                    rsum = stats.tile([TILE_P, 1], f32, tag="rsum")
                    nc.vector.reciprocal(rsum[:], sumexp[:])
                    for bi in range(D // DBLOCK):
                        blk = slice(bi * DBLOCK, (bi + 1) * DBLOCK)
                        for n in range(N_SLICES):
                            nc.tensor.matmul(
                                ps[:, blk],
                                dg[:, n, :],
                                vtiles[n // 2][:, n % 2, blk],
                                start=(n == 0),
                                stop=(n == N_SLICES - 1),
                            )
                        emit_copy(blk, rsum, on_dve=False)
                    # One contiguous 1 MiB store per tile (8 KiB rows).
                    nc.scalar.dma_start(
                        out=OUTap[p0 : p0 + TILE_P, :], in_=o_sb[:]
                    )
                else:
                    # Early matmuls (slices 0..9) run while the final input
                    # pair is still streaming in.
                    for bi in range(D // DBLOCK):
                        blk = slice(bi * DBLOCK, (bi + 1) * DBLOCK)
                        for n in range(n_early):
                            nc.tensor.matmul(
                                ps[:, blk],
                                dg[:, n, :],
                                vtiles[n // 2][:, n % 2, blk],
                                start=(n == 0),
                                stop=False,
                            )
                    n_late = N_SLICES - n_early
                    emit_pair_dma(n_early // 2, split=True)
                    for n in range(n_early, N_SLICES):
                        emit_phase1(n)
                    logitsB = emit_logits(n_early, N_SLICES, "B")
                    # B weights must fit in fp16 diags (< 65504 = e^11.1).
                    # Shift BOTH B logits by adj = relu(max_B - 10): caps
                    # exp at e^10 while keeping the B-vs-B gap exact.  When
                    # the shift engages, the A side's true weights are
                    # <= e^-10 relative, so the scale mismatch it causes
                    # in sum(exp) is negligible (~5e-4).
                    shifted = stats.tile([TILE_P, n_late], f32, tag="shiftB")
                    nc.vector.tensor_scalar(
                        out=shifted[:], in0=logitsB[:], scalar1=negmax[:],
                        scalar2=None, op0=OP.add,
                    )
                    mB = stats.tile([TILE_P, 1], f32, tag="mB")
                    nc.vector.tensor_reduce(
                        mB[:], shifted[:], axis=mybir.AxisListType.X, op=OP.max,
                    )
                    adj = stats.tile([TILE_P, 1], f32, tag="adj")
                    nc.vector.tensor_scalar(
                        out=adj[:], in0=mB[:], scalar1=-10.0, scalar2=0.0,
                        op0=OP.add, op1=OP.max,
                    )
                    nc.vector.tensor_scalar(
                        out=shifted[:], in0=shifted[:], scalar1=adj[:],
                        scalar2=None, op0=OP.subtract,
                    )
                    expB = stats.tile([TILE_P, n_late], f32, tag="expB")
                    sumB = stats.tile([TILE_P, 1], f32, tag="sumB")
                    nc.scalar.activation(
                        expB[:], shifted[:], AF.Exp, bias=zb[:],
                        accum_out=sumB[:],
                    )
                    sumT = stats.tile([TILE_P, 1], f32, tag="sumT")
                    nc.vector.tensor_add(sumT[:], sumexp[:], sumB[:])
                    rsum = stats.tile([TILE_P, 1], f32, tag="rsum")
                    nc.vector.reciprocal(rsum[:], sumT[:])
                    emit_diags(dg, expB, n_early, N_SLICES)
                    for bi in range(D // DBLOCK):
                        blk = slice(bi * DBLOCK, (bi + 1) * DBLOCK)
                        for n in range(n_early, N_SLICES):
                            nc.tensor.matmul(
                                ps[:, blk],
                                dg[:, n, :],
                                vtiles[n // 2][:, n % 2, blk],
                                start=False,
                                stop=(n == N_SLICES - 1),
                            )
                        emit_copy(blk, rsum, on_dve=False)
                        nc.scalar.dma_start(
                            out=OUTap[p0 : p0 + TILE_P, blk], in_=o_sb[:, blk]
                        )

    nc.compile()
    return nc


def get_nc():
    if "nc" not in _CACHE:
        _CACHE["nc"] = _build_module()
    return _CACHE["nc"]


def _shard_inputs(V, norm_scale, query):
    """Full inputs -> per-core input dicts (list of NCORES)."""
    wq = (np.asarray(norm_scale, dtype=np.float32)
          * np.asarray(query, dtype=np.float32)).astype(np.float16)
    wq_rep = np.broadcast_to(wq, (TILE_P, D)).copy()
    zb = np.zeros((TILE_P, 1), dtype=np.float32)
    ident = np.eye(TILE_P, dtype=np.float16)
    Vflat = np.asarray(V).reshape(N_SLICES, B * S, D)
    in_maps = []
    for c in range(NCORES):
        shard = np.ascontiguousarray(
            Vflat[:, c * POS_PER_CORE : (c + 1) * POS_PER_CORE, :],
            dtype=np.float16,
        )
        in_maps.append(
            {"v_in": shard, "wq_in": wq_rep, "id_in": ident, "zb_in": zb}
        )
    return in_maps


def _unshard_output(per_core_outs):
    out = np.empty((B * S, D), dtype=np.float32)
    for c in range(NCORES):
        out[c * POS_PER_CORE : (c + 1) * POS_PER_CORE] = per_core_outs[c]
    return out.reshape(B, S, D)


class _Runner:
    """Jitted 8-core SPMD executor for the bass module.

    Mirrors concourse.bass2jax.run_bass_via_pjrt (exec lowering: the jit body
    must contain only parameters + the bass_exec custom call, with zero
    output buffers passed as donated trailing parameters), but holds the
    jitted callable so repeated invocations don't re-trace/re-compile.
    """

    def __init__(self):
        import jax
        import jax.numpy as jnp
        from jax.sharding import Mesh, PartitionSpec, NamedSharding
        from jax.experimental.shard_map import shard_map
        import concourse.mybir as mybir
        from concourse import bass2jax

        bass2jax.install_neuronx_cc_hook()
        nc = get_nc()
        self._jax = jax

        in_names = []
        out_names = []
        out_avals = []
        for alloc in nc.m.functions[0].allocations:
            if not isinstance(alloc, mybir.MemoryLocationSet):
                continue
            if not alloc.memorylocations:
                continue
            name = alloc.memorylocations[0].name
            if alloc.kind == "ExternalInput":
                in_names.append(name)
            elif alloc.kind == "ExternalOutput":
                out_names.append(name)
                out_avals.append(
                    jax.core.ShapedArray(
                        tuple(alloc.tensor_shape), mybir.dt.np(alloc.dtype)
                    )
                )
        self.in_names = in_names
        self.out_names = out_names
        n_params = len(in_names)
        n_outs = len(out_names)
        all_names = tuple(in_names) + tuple(out_names)

        def _body(*args):
            outs = bass2jax._bass_exec_p.bind(
                *args,
                out_avals=tuple(out_avals),
                in_names=all_names,
                out_names=tuple(out_names),
                lowering_input_output_aliases=(),
                sim_require_finite=True,
                sim_require_nnan=True,
                nc=nc,
            )
            return tuple(outs)

        devices = jax.devices()[:NCORES]
        assert len(devices) == NCORES, f"need {NCORES} cores, got {len(devices)}"
        mesh = Mesh(np.asarray(devices), ("core",))
        self.mesh = mesh
        spec = PartitionSpec("core")
        self.sharding = NamedSharding(mesh, spec)
        in_specs = (spec,) * (n_params + n_outs)
        out_specs = (spec,) * n_outs
        self.fn = jax.jit(
            shard_map(_body, mesh=mesh, in_specs=in_specs, out_specs=out_specs,
                      check_rep=False),
            donate_argnums=tuple(range(n_params, n_params + n_outs)),
            keep_unused=True,
        )
        self.mkzeros = jax.jit(
            lambda: tuple(
                jnp.zeros((NCORES * a.shape[0], *a.shape[1:]), a.dtype)
                for a in out_avals
            ),
            out_shardings=tuple(self.sharding for _ in out_avals),
        )

    def pack(self, in_maps):
        return [
            np.concatenate(
                [np.asarray(in_maps[c][name]) for c in range(NCORES)], axis=0
            )
            for name in self.in_names
        ]

    def put(self, packed):
        return [self._jax.device_put(a, self.sharding) for a in packed]

    def unpack(self, out_arrs):
        arr = np.asarray(out_arrs[self.out_names.index("out")])
        return [arr.reshape(NCORES, POS_PER_CORE, D)[c] for c in range(NCORES)]


def _get_runner():
    if "runner" not in _CACHE:
        _CACHE["runner"] = _Runner()
    return _CACHE["runner"]


def kernel(V, norm_scale, query):
    r = _get_runner()
    in_maps = _shard_inputs(V, norm_scale, query)
    packed = r.put(r.pack(in_maps))
    zeros = r.mkzeros()
    out_arrs = r.fn(*packed, *zeros)
    per_core = r.unpack([np.asarray(a) for a in out_arrs])
    return _unshard_output(per_core)


if __name__ == "__main__":
    # smoke test on random data
    rng = np.random.default_rng(0)
    V = rng.standard_normal((N_SLICES, B, S, D)).astype(np.float32)
    ns = np.ones((D,), dtype=np.float32)
    q = rng.standard_normal((D,)).astype(np.float32)
    out = kernel(V=V, norm_scale=ns, query=q)
    print("out", out.shape, out.dtype, float(np.abs(out).mean()))
